# revision 8
# baseline (speedup 1.0000x reference)
"""BiLSTM-CRF Trainium2 kernel (8 NeuronCores, SPMD).

Strategy:
  - The sequential LSTM recurrence is the bottleneck (batch=1). Per-step
    cross-core collectives are infeasible (~5us floor), so each direction's
    2048 steps are split into 4 chunks of 512 run on separate cores, each
    preceded by a 128-step warmup from zero state (LSTM forget gates decay
    init-state influence below fp32 eps within ~32 steps; verified 2e-8).
    Cores 0-3: forward chunks 0-3. Cores 4-7: backward chunks (the backward
    direction runs the same program on the reversed sequence).
  - Per step, gates = Whh @ h accumulate in PSUM as 16 column vectors
    [128 partitions, 1] (gate dim on partitions) with Whh.T (bf16) as the
    stationary operand. xg = x @ Wih.T is precomputed on-device per chunk.
  - Hidden states (bf16) are AllGather'd; every core computes the output
    projection featsT = Wout @ [hf; hb] + bout; the tiny [16,16] viterbi
    runs on host in fp32 (same op order as the reference).
"""

import sys

sys.path.insert(0, "/opt/trn_rl_repo")

import numpy as np
import ml_dtypes

S = 2048
E = 1024
H = 512
T = 16
NCORES = 8
CHUNK = 512
WARM = 128
NEG = -10000.0
START, STOP = 0, 1

_BF16 = ml_dtypes.bfloat16
WDT = "bf16"  # recurrence weight dtype: "bf16" or "fp8" (x64-scaled e4m3)


def build_nc(E=E, H=H, chunk=CHUNK, warm=WARM, unroll=16, wdt="bf16"):
    """Build the SPMD single-core program. All 8 cores run it; inputs differ."""
    import concourse.bass as bass
    import concourse.tile as tile
    from concourse import mybir, bacc
    from contextlib import ExitStack

    f32 = mybir.dt.float32
    bf16 = mybir.dt.bfloat16
    AF = mybir.ActivationFunctionType
    OP = mybir.AluOpType

    NE = E // 128
    NH = H // 128
    NG = 4 * H // 128
    NB = NCORES // 2  # chunks (blocks) per direction
    L = chunk + warm
    SFULL = NB * chunk

    nc = bacc.Bacc(trn_type="TRN2", num_devices=NCORES)

    xT_in = nc.dram_tensor("xT", [E, L], bf16, kind="ExternalInput")
    wihT_in = nc.dram_tensor("wihT", [E, 4 * H], bf16, kind="ExternalInput")
    wmt = bf16 if wdt == "bf16" else mybir.dt.float8e4
    whhT_in = nc.dram_tensor("whhT", [H, 4 * H], wmt, kind="ExternalInput")
    biasT_in = nc.dram_tensor("biasT", [128, NG], f32, kind="ExternalInput")
    h0_in = nc.dram_tensor("h0", [128, NH], f32, kind="ExternalInput")
    c0_in = nc.dram_tensor("c0", [128, NH], f32, kind="ExternalInput")
    woutT_in = nc.dram_tensor("woutT", [E, T], bf16, kind="ExternalInput")
    boutT_in = nc.dram_tensor("boutT", [T, 1], f32, kind="ExternalInput")
    feats_out = nc.dram_tensor("featsT", [T, SFULL], f32, kind="ExternalOutput")

    HCOLS = NH * (L + 1)
    hs_loc = nc.dram_tensor("hs_loc", [128, 2 * HCOLS], bf16)  # [hsT | hsT_rev]
    hs_all = nc.dram_tensor(
        "hs_all", [NCORES, 128, 2 * HCOLS], bf16, addr_space="Shared"
    )

    with tile.TileContext(nc) as tc:
        with ExitStack() as ctx:
            const = ctx.enter_context(tc.tile_pool(name="const", bufs=1))
            state = ctx.enter_context(tc.tile_pool(name="state", bufs=1))
            work = ctx.enter_context(tc.tile_pool(name="work", bufs=2))
            psum = ctx.enter_context(tc.tile_pool(name="psum", bufs=2, space="PSUM"))
            rec_ps = ctx.enter_context(tc.tile_pool(name="rec", bufs=2, space="PSUM"))
            fps = ctx.enter_context(tc.tile_pool(name="fps", bufs=2, space="PSUM"))

            # ---- constants ----
            wihT = [const.tile([128, 4 * H], bf16, tag=f"wihT{k}", name=f"wihT{k}") for k in range(NE)]
            for k in range(NE):
                nc.sync.dma_start(wihT[k][:], wihT_in[128 * k : 128 * (k + 1), :])
            whhT = [const.tile([128, 4 * H], wmt, tag=f"whhT{j}", name=f"whhT{j}") for j in range(NH)]
            for j in range(NH):
                nc.sync.dma_start(whhT[j][:], whhT_in[128 * j : 128 * (j + 1), :])
            xT = [const.tile([128, L], bf16, tag=f"xT{k}", name=f"xTs{k}") for k in range(NE)]
            for k in range(NE):
                nc.sync.dma_start(xT[k][:], xT_in[128 * k : 128 * (k + 1), :])
            biasT = const.tile([128, NG], f32)
            nc.sync.dma_start(biasT[:], biasT_in[:])
            woutT = [const.tile([128, T], bf16, tag=f"woutT{k}", name=f"woutTs{k}") for k in range(NE)]
            for k in range(NE):
                nc.sync.dma_start(woutT[k][:], woutT_in[128 * k : 128 * (k + 1), :])
            boutT = const.tile([T, 1], f32)
            nc.sync.dma_start(boutT[:], boutT_in[:])

            # ---- state ----
            hsT = state.tile([128, HCOLS], bf16)  # block b = h after step b-1
            hsRv = state.tile([128, HCOLS], bf16)  # reversed-position copy
            c_cur = state.tile([128, NH], f32)
            h0sb = state.tile([128, NH], f32)
            nc.sync.dma_start(h0sb[:], h0_in[:])
            nc.sync.dma_start(c_cur[:], c0_in[:])
            nc.vector.tensor_copy(hsT[:, 0:NH], h0sb[:])  # cast to bf16
            nc.vector.memset(hsRv[:, 0:NH], 0.0)  # block 0 never written

            # ---- xgT = (x @ Wih.T).T + bias, gate tiles as strided cols ----
            xgT = state.tile([128, NG * L], bf16)
            NT = 320 if L % 320 == 0 else L
            assert L % NT == 0 and NT <= 512
            for m in range(NG):
                for n in range(L // NT):
                    ps = psum.tile([128, NT], f32, tag="xg_ps")
                    for k in range(NE):
                        nc.tensor.matmul(
                            ps[:],
                            wihT[k][:, 128 * m : 128 * (m + 1)],
                            xT[k][:, NT * n : NT * (n + 1)],
                            start=(k == 0),
                            stop=(k == NE - 1),
                        )
                    o = NG * NT * n + m
                    nc.vector.tensor_scalar_add(
                        xgT[:, o : o + NG * (NT - 1) + 1 : NG], ps[:], biasT[:, m : m + 1]
                    )

            # ---- recurrence ----
            def step(l):
                pg = rec_ps.tile([128, NG], f32, tag="pg")
                for j in range(NH):
                    rhs = hsT[:, bass.ds(NH * l + j, 1)]
                    for m in range(NG):
                        nc.tensor.matmul(
                            pg[:, m : m + 1],
                            whhT[j][:, 128 * m : 128 * (m + 1)],
                            rhs,
                            start=(j == 0),
                            stop=(j == NH - 1),
                        )
                gates = work.tile([128, NG], f32, tag="gates")
                if wdt == "bf16":
                    nc.vector.tensor_tensor(
                        out=gates[:], in0=pg[:], in1=xgT[:, bass.ds(NG * l, NG)],
                        op=OP.add,
                    )
                else:  # fp8 weights are scaled by 64; undo while adding xg
                    nc.vector.scalar_tensor_tensor(
                        out=gates[:], in0=pg[:], scalar=1.0 / 64.0,
                        in1=xgT[:, bass.ds(NG * l, NG)], op0=OP.mult, op1=OP.add,
                    )
                ga = work.tile([128, NG], f32, tag="ga")
                nc.scalar.activation(ga[:, : 2 * NH], gates[:, : 2 * NH], AF.Sigmoid)
                nc.scalar.activation(
                    ga[:, 2 * NH : 3 * NH], gates[:, 2 * NH : 3 * NH], AF.Tanh
                )
                nc.scalar.activation(
                    ga[:, 3 * NH :], gates[:, 3 * NH :], AF.Sigmoid
                )
                t1 = work.tile([128, NH], f32, tag="t1")
                nc.vector.tensor_mul(t1[:], ga[:, 0:NH], ga[:, 2 * NH : 3 * NH])
                t2 = work.tile([128, NH], f32, tag="t2")
                nc.vector.tensor_mul(t2[:], ga[:, NH : 2 * NH], c_cur[:])
                nc.vector.tensor_add(c_cur[:], t1[:], t2[:])
                tct = work.tile([128, NH], f32, tag="tct")
                nc.scalar.activation(tct[:], c_cur[:], AF.Tanh)
                # write h directly into hsT (bf16) so the next step's matmuls
                # unblock immediately; the hsRv copy is off the critical path
                nc.vector.tensor_mul(
                    hsT[:, bass.ds(NH * (l + 1), NH)], ga[:, 3 * NH :], tct[:]
                )
                nc.vector.tensor_copy(
                    hsRv[:, bass.ds(NH * L - NH * l, NH)],
                    hsT[:, bass.ds(NH * (l + 1), NH)],
                )

            tc.For_i_unrolled(0, L, 1, step, max_unroll=unroll)

            # ---- AllGather hidden states ----
            nc.sync.dma_start(hs_loc[:, 0:HCOLS], hsT[:])
            nc.sync.dma_start(hs_loc[:, HCOLS:], hsRv[:])
            nc.gpsimd.collective_compute(
                "AllGather",
                OP.bypass,
                replica_groups=[list(range(NCORES))],
                ins=[hs_loc[:, :]],
                outs=[hs_all[:, :, :]],
            )
            # fwd shards need the hsT half; bwd shards the hsRv half
            shf, shb = [], []
            for r in range(NB):
                t = state.tile([128, HCOLS], bf16, tag=f"shf{r}", name=f"shf{r}")
                nc.sync.dma_start(t[:], hs_all[r, :, 0:HCOLS])
                shf.append(t)
            for r in range(NB):
                t = state.tile([128, HCOLS], bf16, tag=f"shb{r}", name=f"shb{r}")
                nc.sync.dma_start(t[:], hs_all[NB + r, :, HCOLS:])
                shb.append(t)

            # ---- featsT = Wout @ [hf; hb] + bout ----
            featsT = state.tile([T, SFULL], f32)
            for r in range(NB):
                pf = fps.tile([T, chunk], f32, tag="pf")
                l0f = (0 if r == 0 else warm) + 1  # hsT block of step t=chunk*r
                for j in range(NH):
                    st = NH * l0f + j
                    nc.tensor.matmul(
                        pf[:],
                        woutT[j][:, :],
                        shf[r][:, st : st + NH * (chunk - 1) + 1 : NH],
                        start=(j == 0),
                        stop=False,
                    )
                # bwd chunk covering block r is core NB+(NB-1-r); hsRv cols ascend
                l0r = 1 if r < NB - 1 else warm + 1
                sb_ = shb[NB - 1 - r]
                for j in range(NH):
                    st = NH * l0r + j
                    nc.tensor.matmul(
                        pf[:],
                        woutT[NH + j][:, :],
                        sb_[:, st : st + NH * (chunk - 1) + 1 : NH],
                        start=False,
                        stop=(j == NH - 1),
                    )
                nc.vector.tensor_scalar_add(
                    featsT[:, chunk * r : chunk * (r + 1)], pf[:], boutT[:]
                )
            nc.sync.dma_start(feats_out[:], featsT[:])

    nc.finalize()
    return nc


def prep_inputs(inputs, E=E, H=H, chunk=CHUNK, warm=WARM, wdt="bf16"):
    """Host-side sharding/layout prep. Returns per-core in_maps."""
    sent = np.asarray(inputs["sentence"]).astype(np.int64)
    emb = np.asarray(inputs["emb"], dtype=np.float32)
    h0 = np.asarray(inputs["h0"], dtype=np.float32)
    c0 = np.asarray(inputs["c0"], dtype=np.float32)
    NB = NCORES // 2
    L = chunk + warm

    def colmaj(v):  # [n*128] -> [128, n], v[j*128+p] at [p, j]
        return np.ascontiguousarray(v.reshape(-1, 128).T)

    in_maps = []
    for c in range(NCORES):
        d = "f" if c < NB else "b"
        k = c % NB
        seq = sent if d == "f" else sent[::-1]
        lo = 0 if k == 0 else chunk * k - warm
        x = emb[seq[lo : lo + L]]  # [L, E]
        Wih = np.asarray(inputs[f"Wih_{d}"], dtype=np.float32)
        Whh = np.asarray(inputs[f"Whh_{d}"], dtype=np.float32)
        bias = np.asarray(inputs[f"bih_{d}"], dtype=np.float32) + np.asarray(
            inputs[f"bhh_{d}"], dtype=np.float32
        )
        Wout = np.asarray(inputs["Wout"], dtype=np.float32)
        zi = 0 if d == "f" else 1
        h0c = h0[zi] if k == 0 else np.zeros(H, np.float32)
        c0c = c0[zi] if k == 0 else np.zeros(H, np.float32)
        in_maps.append(
            {
                "xT": np.ascontiguousarray(x.T).astype(_BF16),
                "wihT": np.ascontiguousarray(Wih.T).astype(_BF16),
                "whhT": (
                    np.ascontiguousarray(Whh.T).astype(_BF16)
                    if wdt == "bf16"
                    else np.ascontiguousarray(Whh.T * 64.0).astype(
                        ml_dtypes.float8_e4m3
                    )
                ),
                "biasT": colmaj(bias),
                "h0": colmaj(h0c),
                "c0": colmaj(c0c),
                "woutT": np.ascontiguousarray(Wout.T).astype(_BF16),
                "boutT": np.asarray(inputs["bout"], np.float32).reshape(T, 1),
            }
        )
    return in_maps


def viterbi_host(featsT, transitions):
    """Exact replica of the reference viterbi in fp32 numpy."""
    feats = np.ascontiguousarray(featsT.T)  # [S, T]
    trans = np.asarray(transitions, dtype=np.float32)
    n = feats.shape[0]
    v = np.full(T, NEG, np.float32)
    v[START] = 0.0
    vs = np.empty((n, T), np.float32)
    for t in range(n):
        v = (v[None, :] + trans).max(axis=1) + feats[t]
        vs[t] = v
    terminal = vs[n - 1] + trans[STOP]
    best = int(np.argmax(terminal))
    path = np.empty(n, np.int32)
    path[n - 1] = best
    for t in range(n - 2, -1, -1):
        path[t] = np.argmax(vs[t] + trans[path[t + 1]])
    return np.float32(terminal[best]), path


_NC_CACHE = {}


def kernel(**inputs):
    from concourse.bass_utils import run_bass_kernel_spmd

    if "nc" not in _NC_CACHE:
        _NC_CACHE["nc"] = build_nc(wdt=WDT)
    nc = _NC_CACHE["nc"]
    in_maps = prep_inputs(inputs, wdt=WDT)
    res = run_bass_kernel_spmd(nc, in_maps, core_ids=list(range(NCORES)))
    featsT = np.asarray(res.results[0]["featsT"], dtype=np.float32)
    return viterbi_host(featsT, inputs["transitions"])
